# revision 27
# baseline (speedup 1.0000x reference)
"""Capsule-routing kernel for 8 Trainium2 NeuronCores.

Problem: u_hat = einsum('nidk,bik->bnid', W, x); 3 rounds of dynamic
routing (softmax over n, weighted sum over i, squash, agreement update).

Sharding: input-capsule axis i (2048) split 8 ways -> 256 i per core.
Per-iteration partial sums s[b,n,d] are combined with an on-device
fp16 AllReduce (iters 0,1) or on the host (final iteration).

v2 design (from ~723us baseline):
  - s0 (iteration-0 weighted sum; c is uniform) is computed DIRECTLY
    from x,W on PE: s0[b,(d,n)] = sum_{i,k} x8[(i8,k),b]*W[(i8,k),(d,n)].
  - W is streamed twice (pass 1 for s0, pass 2 for u_hat).
  - Iter 1 consumes u_hat pairs DIRECTLY from the production SBUF tiles
    (unified pair pool: RES_P pinned + ROT rotating slots); the spill to
    DRAM happens write-behind and is only read back for iter 2.
  - squash is never materialized on device: v = squash(s)*INV_LOG2 =
    s * fac[b,n], so we load s 4-replicated ([128,dn]) from the AR
    output and compute fac with a short 128-partition tree + smalls,
    then one broadcast mul -> orep.  (~5us vs ~13us for the old
    32-partition full squash; also kills the frep/o16 ops.)
  - Latency-critical DMAs (cc_in writes, s_rep loads) ride the Scalar
    HWDGE ring, away from the bulk W/u traffic on the Sync ring.
  - All elementwise work on DVE plain tensor_tensor fp16 (2x mode);
    agreement = tmp mul + 4-stage halving tree over d; psum drains on
    ACT; softmax max negated via tensor_reduce feeds exp bias; fold via
    cz4 ones-matmul accumulated across groups into one psum tile.
  - Layouts: u16 partition p = 32*j + b (j = i mod 4 in group), free
    (d,n) d-major.
"""
import sys
import types

sys.path.insert(0, "/opt/trn_rl_repo")

import numpy as np

from concourse import bacc, tile, mybir
from concourse.bass_utils import run_bass_kernel_spmd

f32 = mybir.dt.float32
f16 = mybir.dt.float16
f8 = mybir.dt.float8e4
AX = mybir.AxisListType
OP = mybir.AluOpType
AF = mybir.ActivationFunctionType

B, N, I, D, K = 32, 64, 2048, 32, 16
NCORES = 8
IL = I // NCORES          # 256 local input capsules
G = IL // 4               # 64 groups of 4 i
NP = G // 2               # 32 group-pairs
DN = D * N                # 2048 free elements per group, d-major
INV_LOG2 = float(1.0 / np.log(2.0))

RES_P = 2                 # pairs kept SBUF-resident; rest spilled to DRAM
ROT = 7                   # rotating pair slots
LAG = 2                   # iter1 consumes pair q at step q+LAG


def _install_ntff_hook():
    if "antenv.axon_hooks" in sys.modules:
        return
    try:
        mod = types.ModuleType("antenv.axon_hooks")
        state = {"hook": None}
        mod.set_axon_ntff_profile_hook = lambda h: state.__setitem__("hook", h)
        mod.get_axon_ntff_profile_hook = lambda: state["hook"]
        sys.modules["antenv.axon_hooks"] = mod
        import antenv
        antenv.axon_hooks = mod
        from trn_agent_boot.trn_boot import _ntff_profile_via_ctypes
        mod.set_axon_ntff_profile_hook(
            _ntff_profile_via_ctypes("/opt/axon/libaxon_pjrt.so"))
    except Exception:
        pass


def _build():
    nc = bacc.Bacc("TRN2", target_bir_lowering=False, debug=False,
                   num_devices=NCORES)

    w_t2 = nc.dram_tensor("w_t2", [NP, 128, DN], f16, kind="ExternalInput")
    x_bd = nc.dram_tensor("x_bd", [128, NP, 128], f16, kind="ExternalInput")
    x8 = nc.dram_tensor("x8", [128, NP, B], f16, kind="ExternalInput")
    s2_part = nc.dram_tensor("s2_part", [B, DN], f32, kind="ExternalOutput")

    u_spill = nc.dram_tensor("u_spill", [G, 128, DN], f16)
    cc_in = [nc.dram_tensor(f"cc_in{r}", [B, DN], f16) for r in range(2)]
    cc_out = [nc.dram_tensor(f"cc_out{r}", [B, DN], f16, addr_space="Shared")
              for r in range(2)]

    ones4_np = np.zeros((128, 32), np.float16)
    for p in range(128):
        ones4_np[p, p % 32] = 1.0
    ones4 = nc.inline_tensor(ones4_np, name="ones4")

    core_ids = list(range(NCORES))

    with tile.TileContext(nc) as tc:
        with tc.tile_pool(name="const", bufs=1) as constp, \
             tc.tile_pool(name="sqp", bufs=1) as sqp, \
             tc.tile_pool(name="drp", bufs=1) as drp, \
             tc.tile_pool(name="xw", bufs=1) as xw, \
             tc.tile_pool(name="wp", bufs=4) as wp, \
             tc.tile_pool(name="upool", bufs=1) as upool, \
             tc.tile_pool(name="tmpp", bufs=2) as tmpp, \
             tc.tile_pool(name="smp", bufs=2) as smp, \
             tc.tile_pool(name="tr1p", bufs=2) as tr1p, \
             tc.tile_pool(name="tr2p", bufs=2) as tr2p, \
             tc.tile_pool(name="tr3p", bufs=2) as tr3p, \
             tc.tile_pool(name="t4p", bufs=2) as t4p, \
             tc.tile_pool(name="bstate", bufs=1) as bstate, \
             tc.tile_pool(name="small", bufs=4) as small, \
             tc.tile_pool(name="psu", bufs=2, space="PSUM") as psu, \
             tc.tile_pool(name="psacc", bufs=1, space="PSUM") as psacc:

            ones_sb = constp.tile([128, 32], f16)
            nc.sync.dma_start(ones_sb[:], ones4[:])
            xbd_sb = xw.tile([128, NP, 128], f16)
            nc.sync.dma_start(xbd_sb[:], x_bd[:])
            x8_sb = xw.tile([128, NP, B], f16)
            nc.scalar.dma_start(x8_sb[:], x8[:])
            orep = [constp.tile([128, DN], f16, tag=f"orep{r}",
                                name=f"orep{r}") for r in range(2)]

            def scale_to_orep(orep_t, cc_out_list, pre_scale):
                """orep <- s * fac with fac[b,n] chosen so that
                orep = squash(pre_scale*s)*INV_LOG2, all on 128-part
                tiles; s loaded 4-replicated from the AR output(s)."""
                s_rep = sqp.tile([128, DN], f16, tag="q_srep")
                for j in range(4):
                    nc.scalar.dma_start(s_rep[32 * j:32 * j + 32, :],
                                        cc_out_list[0][:])
                if len(cc_out_list) > 1:
                    s_rep2 = sqp.tile([128, DN], f16, tag="q_srep2")
                    for j in range(4):
                        nc.scalar.dma_start(s_rep2[32 * j:32 * j + 32, :],
                                            cc_out_list[1][:])
                    s_sum = sqp.tile([128, DN], f16, tag="q_ssum")
                    nc.vector.tensor_add(s_sum[:], s_rep[:], s_rep2[:])
                    s_rep = s_sum
                # sq = (pre*s)^2 via the ACT affine (keeps fp16 in range)
                sq = sqp.tile([128, D, N], f16, tag="q_sq")
                nc.scalar.activation(
                    sq[:], s_rep[:].rearrange("p (d n) -> p d n", n=N),
                    AF.Square, bias=0.0, scale=float(pre_scale))
                cur, d = sq, D
                while d > 1:
                    nxt = sqp.tile([128, d // 2, N], f16, tag=f"q_t{d}")
                    nc.vector.tensor_add(nxt[:], cur[:, 0:d // 2, :],
                                         cur[:, d // 2:d, :])
                    cur, d = nxt, d // 2
                r_ = sqp.tile([128, N], f32, tag="q_r")
                nc.scalar.activation(r_[:], cur[:, 0, :], AF.Sqrt,
                                     bias=0.0, scale=1.0)
                den = sqp.tile([128, N], f32, tag="q_den")
                nc.vector.tensor_scalar(den[:], cur[:, 0, :], 1.0, 1.0,
                                        OP.mult, OP.add)
                rd = sqp.tile([128, N], f32, tag="q_rd")
                nc.vector.reciprocal(rd[:], den[:])
                fac = sqp.tile([128, N], f16, tag="q_fac")
                nc.vector.scalar_tensor_tensor(fac[:], r_[:],
                                               float(pre_scale * INV_LOG2),
                                               rd[:],
                                               op0=OP.mult, op1=OP.mult)
                nc.vector.tensor_mul(
                    orep_t[:].rearrange("p (d n) -> p d n", n=N),
                    s_rep[:].rearrange("p (d n) -> p d n", n=N),
                    fac[:].unsqueeze(1).broadcast_to([128, D, N]))

            # =========== pass 1: s0 directly from x,W ===========
            s0_ps = psacc.tile([B, DN], f32, tag="sacc")
            for gp in range(NP):
                wt = wp.tile([128, DN], f16, tag="wt")
                nc.sync.dma_start(wt[:], w_t2[gp])
                for ch in range(4):
                    nc.tensor.matmul(
                        s0_ps[:, 512 * ch:512 * (ch + 1)],
                        lhsT=x8_sb[:, gp, :],
                        rhs=wt[:, 512 * ch:512 * (ch + 1)],
                        start=(gp == 0), stop=(gp == NP - 1),
                        skip_group_check=True)
            s0_dr = drp.tile([B, DN], f16, tag="t_io16")
            nc.scalar.copy(s0_dr[:], s0_ps[:])
            nc.scalar.dma_start(cc_in[0][:], s0_dr[:])
            nc.gpsimd.collective_compute(
                "AllReduce", OP.add, ins=[cc_in[0][:]],
                outs=[cc_out[0][:]], replica_groups=[core_ids])
            scale_to_orep(orep[0], [cc_out[0]], 1.0 / 64.0)

            # =========== merged: u_hat production (pass 2) + iter 1 ===========
            pair_tiles = {}    # pair id -> sbuf tile [128, 2, DN]
            bs_tiles = {}

            def pair_tile(gp):
                if gp < RES_P:
                    return upool.tile([128, 2, DN], f16, tag=f"ur{gp}",
                                      name=f"ur{gp}", bufs=1)
                return upool.tile([128, 2, DN], f16, tag="rot", name="rot",
                                  bufs=ROT)

            def emit_produce(gp):
                """Pass-2 W load + u_hat matmuls + drains for pair gp."""
                wt = wp.tile([128, DN], f16, tag="wt")
                nc.sync.dma_start(wt[:], w_t2[gp])
                pt = pair_tile(gp)
                pair_tiles[gp] = pt
                for gs in range(2):
                    for h in range(2):
                        pu = psu.tile([128, DN // 2], f32)
                        for ch in range(2):
                            nc.tensor.matmul(
                                pu[:, 512 * ch:512 * (ch + 1)],
                                lhsT=xbd_sb[64 * gs:64 * (gs + 1), gp, :],
                                rhs=wt[64 * gs:64 * (gs + 1),
                                       1024 * h + 512 * ch:
                                       1024 * h + 512 * (ch + 1)],
                                start=True, stop=True)
                        nc.scalar.copy(pt[:, gs, 1024 * h:1024 * (h + 1)],
                                       pu[:])
                if gp >= RES_P:
                    nc.sync.dma_start(
                        u_spill[2 * gp:2 * gp + 2].transpose([1, 0, 2]), pt[:])

            def load_pair(q):
                """DRAM -> rotating tile for spilled pair q (iter 2)."""
                pt = upool.tile([128, 2, DN], f16, tag="rot", name="rot",
                                bufs=ROT)
                nc.sync.dma_start(
                    pt[:], u_spill[2 * q:2 * q + 2].transpose([1, 0, 2]))
                return pt

            st = {}

            def it_stage_a(q, u_pt, orep_t):
                """tmp mul + DVE tree stages (d 32 -> 4)."""
                u3 = u_pt[:]                                  # [128, 2, 2048]
                orep3 = orep_t[:].unsqueeze(1).broadcast_to([128, 2, DN])
                tmp = tmpp.tile([128, 2, DN], f16, tag="tmp")
                nc.vector.tensor_mul(tmp[:], u3, orep3)
                t1 = tr1p.tile([128, 2, 16 * N], f16, tag="t1")
                nc.vector.tensor_add(t1[:], tmp[:, :, 0:16 * N],
                                     tmp[:, :, 16 * N:32 * N])
                t2 = tr2p.tile([128, 2, 8 * N], f16, tag="t2")
                nc.vector.tensor_add(t2[:], t1[:, :, 0:8 * N],
                                     t1[:, :, 8 * N:16 * N])
                t3 = tr3p.tile([128, 2, 4 * N], f16, tag="t3")
                nc.vector.tensor_add(t3[:], t2[:, :, 0:4 * N],
                                     t2[:, :, 4 * N:8 * N])
                st[q] = {"u3": u3, "t3": t3}

            def it_stage_b(q, first_iter):
                """DVE tree tail + bs update + max; ACT exp."""
                t3 = st[q]["t3"]
                t4 = t4p.tile([128, 2, 2 * N], f16, tag="t4")
                nc.vector.tensor_add(t4[:], t3[:, :, 0:2 * N],
                                     t3[:, :, 2 * N:4 * N])
                if first_iter:
                    bs = bstate.tile([128, 2, N], f32, tag=f"bs{q}",
                                     name=f"bs{q}")
                    bs_tiles[q] = bs
                    nc.vector.tensor_add(bs[:], t4[:, :, 0:N], t4[:, :, N:2 * N])
                else:
                    bs = bs_tiles[q]
                    a2 = small.tile([128, 2, N], f32, tag="a2")
                    nc.vector.tensor_add(a2[:], t4[:, :, 0:N], t4[:, :, N:2 * N])
                    nc.vector.tensor_add(bs[:], bs[:], a2[:])
                nm2 = small.tile([128, 2, 1], f32, tag="nm2")
                nc.vector.tensor_reduce(out=nm2[:], in_=bs[:], axis=AX.X,
                                        op=OP.max, negate=True)
                e2 = small.tile([128, 2, N], f16, tag="e2")
                for gs in range(2):
                    nc.scalar.activation(e2[:, gs, :], bs[:, gs, :], AF.Exp,
                                         bias=nm2[:, gs, :], scale=1.0)
                st[q].update(e2=e2)

            def it_stage_c(q, s_ps, first_mm, last_mm):
                """DVE rz + sm mul; ACT cz4; PE folds."""
                d = st.pop(q)
                z2 = small.tile([128, 2, 1], f32, tag="z2")
                nc.vector.tensor_reduce(out=z2[:], in_=d["e2"][:], axis=AX.X,
                                        op=OP.add)
                rz = small.tile([128, 2, 1], f32, tag="rz")
                nc.vector.reciprocal(rz[:], z2[:])
                cz4 = small.tile([128, 2, 32], f16, tag="cz4")
                for gs in range(2):
                    nc.scalar.activation(cz4[:, gs, :], ones_sb[:], AF.Copy,
                                         bias=0.0, scale=rz[:, gs, :])
                sm = smp.tile([128, 2, DN], f16, tag="sm")
                u4 = d["u3"].rearrange("p a (d n) -> p a d n", n=N)
                e4 = d["e2"][:].unsqueeze(2).broadcast_to([128, 2, D, N])
                # offload some sm muls to GPSIMD: sm feeds only the PE
                # folds, so the slower engine stays off the DVE chain
                eng = nc.gpsimd if (q % 2 == 1) else nc.vector
                eng.tensor_mul(
                    sm[:].rearrange("p a (d n) -> p a d n", n=N), u4, e4)
                for gs in range(2):
                    for ch in range(4):
                        nc.tensor.matmul(
                            s_ps[:, 512 * ch:512 * (ch + 1)],
                            lhsT=cz4[:, gs, :],
                            rhs=sm[:, gs, 512 * ch:512 * (ch + 1)],
                            start=(first_mm and gs == 0),
                            stop=(last_mm and gs == 1),
                            skip_group_check=True)

            s1_ps = psacc.tile([B, DN], f32, tag="sacc")
            for step in range(NP + LAG + 2):
                if step < NP:
                    emit_produce(step)
                pa = step - LAG
                if 0 <= pa < NP:
                    it_stage_a(pa, pair_tiles[pa], orep[0])
                pb = pa - 1
                if 0 <= pb < NP:
                    it_stage_b(pb, first_iter=True)
                pc = pa - 2
                if 0 <= pc < NP:
                    it_stage_c(pc, s1_ps, first_mm=(pc == 0),
                               last_mm=(pc == NP - 1))
                    if pc >= RES_P:
                        pair_tiles.pop(pc)

            # iter-1 tail: AllReduce -> orep1
            s1_dr = drp.tile([B, DN], f16, tag="t_io16")
            nc.scalar.copy(s1_dr[:], s1_ps[:])
            nc.scalar.dma_start(cc_in[1][:], s1_dr[:])
            nc.gpsimd.collective_compute(
                "AllReduce", OP.add, ins=[cc_in[1][:]],
                outs=[cc_out[1][:]], replica_groups=[core_ids])
            # prefetch first spilled pairs for iter 2 during the AR
            it2_tiles = {}
            for w in range(RES_P, RES_P + 3):
                it2_tiles[w] = load_pair(w)
            scale_to_orep(orep[1], [cc_out[1]], 1.0)

            # =========== iter 2 (2-stage emission skew) ===========
            s2_ps = psacc.tile([B, DN], f32, tag="sacc")
            for step in range(NP + 2):
                pa = step
                if pa < NP:
                    for w in range(max(pa, RES_P), min(pa + 3, NP)):
                        if w not in it2_tiles:
                            it2_tiles[w] = load_pair(w)
                    if pa < RES_P:
                        u_pt = pair_tiles[pa]
                    else:
                        u_pt = it2_tiles.pop(pa)
                    it_stage_a(pa, u_pt, orep[1])
                pb = step - 1
                if 0 <= pb < NP:
                    it_stage_b(pb, first_iter=False)
                pc = step - 2
                if 0 <= pc < NP:
                    it_stage_c(pc, s2_ps, first_mm=(pc == 0),
                               last_mm=(pc == NP - 1))

            s2_dr = drp.tile([B, DN], f32, tag="t_io")
            nc.scalar.copy(s2_dr[:], s2_ps[:])
            nc.sync.dma_start(s2_part[:], s2_dr[:])

    nc.compile()
    return nc


_NC_CACHE = {}


def _get_nc():
    if "nc" not in _NC_CACHE:
        _NC_CACHE["nc"] = _build()
    return _NC_CACHE["nc"]


def _prep_core(x_c, w_c):
    """x_c [B, IL, K] f32, w_c [N, IL, D, K] f32 -> in_map dict."""
    wt = np.ascontiguousarray(w_c.transpose(1, 3, 2, 0))  # [IL, K, D, N]
    wt2 = wt.reshape(NP, 8, K, DN).reshape(NP, 128, DN).astype(np.float16)
    xt = x_c.transpose(1, 2, 0)  # [IL, K, B]
    x_bd = np.zeros((128, NP, 128), np.float16)
    for g in range(G):
        q, s = g // 2, g % 2
        for j in range(4):
            i = 4 * g + j
            x_bd[s * 64 + j * 16:s * 64 + j * 16 + K, q,
                 j * 32:j * 32 + 32] = xt[i].astype(np.float16)
    # x8[(i8,k), gp, b] = x[b, i, k] for i = gp*8 + i8
    x8 = np.ascontiguousarray(
        xt.reshape(NP, 8 * K, B).transpose(1, 0, 2)).astype(np.float16)
    return {"w_t2": wt2, "x_bd": x_bd, "x8": x8}


def _squash_np(v):
    sn = np.sum(v * v, axis=-1, keepdims=True)
    return np.sqrt(sn) / (1.0 + sn) * v


def _run(inputs, W, trace=False):
    _install_ntff_hook()
    nc = _get_nc()
    x = np.asarray(inputs, np.float32)
    Wf = np.asarray(W, np.float32)
    in_maps = []
    for c in range(NCORES):
        sl = slice(c * IL, (c + 1) * IL)
        in_maps.append(_prep_core(x[:, sl, :], Wf[:, sl, :, :]))
    res = run_bass_kernel_spmd(nc, in_maps, list(range(NCORES)), trace=trace)
    s2 = np.zeros((B, DN), np.float64)
    for c in range(NCORES):
        s2 += res.results[c]["s2_part"].astype(np.float64)
    s2 = s2.reshape(B, D, N).transpose(0, 2, 1).astype(np.float32)
    out = _squash_np(s2).astype(np.float32)
    return out, res


def kernel(inputs, W):
    out, _ = _run(inputs, W, trace=False)
    return out


# revision 28
# speedup vs baseline: 1.2307x; 1.2307x over previous
"""Capsule-routing kernel for 8 Trainium2 NeuronCores.

Problem: u_hat = einsum('nidk,bik->bnid', W, x); 3 rounds of dynamic
routing (softmax over n, weighted sum over i, squash, agreement update).

Sharding: input-capsule axis i (2048) split 8 ways -> 256 i per core.
Per-iteration partial sums s[b,n,d] are combined with an on-device
fp16 AllReduce (iters 0,1) or on the host (final iteration).

v2 design (from ~723us baseline):
  - s0 (iteration-0 weighted sum; c is uniform) is computed DIRECTLY
    from x,W on PE: s0[b,(d,n)] = sum_{i,k} x8[(i8,k),b]*W[(i8,k),(d,n)].
  - W is streamed twice (pass 1 for s0, pass 2 for u_hat).
  - Iter 1 consumes u_hat pairs DIRECTLY from the production SBUF tiles
    (unified pair pool: RES_P pinned + ROT rotating slots); the spill to
    DRAM happens write-behind and is only read back for iter 2.
  - squash is never materialized on device: v = squash(s)*INV_LOG2 =
    s * fac[b,n], so we load s 4-replicated ([128,dn]) from the AR
    output and compute fac with a short 128-partition tree + smalls,
    then one broadcast mul -> orep.  (~5us vs ~13us for the old
    32-partition full squash; also kills the frep/o16 ops.)
  - Latency-critical DMAs (cc_in writes, s_rep loads) ride the Scalar
    HWDGE ring, away from the bulk W/u traffic on the Sync ring.
  - All elementwise work on DVE plain tensor_tensor fp16 (2x mode);
    agreement = tmp mul + 4-stage halving tree over d; psum drains on
    ACT; softmax max negated via tensor_reduce feeds exp bias; fold via
    cz4 ones-matmul accumulated across groups into one psum tile.
  - Layouts: u16 partition p = 32*j + b (j = i mod 4 in group), free
    (d,n) d-major.
"""
import sys
import types

sys.path.insert(0, "/opt/trn_rl_repo")

import numpy as np

from concourse import bacc, tile, mybir
from concourse.bass_utils import run_bass_kernel_spmd

f32 = mybir.dt.float32
f16 = mybir.dt.float16
f8 = mybir.dt.float8e4
AX = mybir.AxisListType
OP = mybir.AluOpType
AF = mybir.ActivationFunctionType

B, N, I, D, K = 32, 64, 2048, 32, 16
NCORES = 8
IL = I // NCORES          # 256 local input capsules
G = IL // 4               # 64 groups of 4 i
NP = G // 2               # 32 group-pairs
DN = D * N                # 2048 free elements per group, d-major
INV_LOG2 = float(1.0 / np.log(2.0))

RES_P = 2                 # pairs kept SBUF-resident; rest spilled to DRAM
ROT = 7                   # rotating pair slots
LAG = 2                   # iter1 consumes pair q at step q+LAG


def _install_ntff_hook():
    if "antenv.axon_hooks" in sys.modules:
        return
    try:
        mod = types.ModuleType("antenv.axon_hooks")
        state = {"hook": None}
        mod.set_axon_ntff_profile_hook = lambda h: state.__setitem__("hook", h)
        mod.get_axon_ntff_profile_hook = lambda: state["hook"]
        sys.modules["antenv.axon_hooks"] = mod
        import antenv
        antenv.axon_hooks = mod
        from trn_agent_boot.trn_boot import _ntff_profile_via_ctypes
        mod.set_axon_ntff_profile_hook(
            _ntff_profile_via_ctypes("/opt/axon/libaxon_pjrt.so"))
    except Exception:
        pass


def _build():
    nc = bacc.Bacc("TRN2", target_bir_lowering=False, debug=False,
                   num_devices=NCORES)

    w_t2 = nc.dram_tensor("w_t2", [NP, 128, DN], f16, kind="ExternalInput")
    x_bd = nc.dram_tensor("x_bd", [128, NP, 128], f16, kind="ExternalInput")
    x8 = nc.dram_tensor("x8", [128, NP, B], f16, kind="ExternalInput")
    s2_part = nc.dram_tensor("s2_part", [B, DN], f32, kind="ExternalOutput")

    u_spill = nc.dram_tensor("u_spill", [G, 128, DN], f16)
    cc_in = [nc.dram_tensor(f"cc_in{r}", [B, DN], f16) for r in range(2)]
    cc_out = [nc.dram_tensor(f"cc_out{r}", [B, DN], f16, addr_space="Shared")
              for r in range(2)]

    ones4_np = np.zeros((128, 32), np.float16)
    for p in range(128):
        ones4_np[p, p % 32] = 1.0
    ones4 = nc.inline_tensor(ones4_np, name="ones4")

    core_ids = list(range(NCORES))

    with tile.TileContext(nc) as tc:
        with tc.tile_pool(name="const", bufs=1) as constp, \
             tc.tile_pool(name="sqp", bufs=1) as sqp, \
             tc.tile_pool(name="drp", bufs=1) as drp, \
             tc.tile_pool(name="xw", bufs=1) as xw, \
             tc.tile_pool(name="wp", bufs=4) as wp, \
             tc.tile_pool(name="upool", bufs=1) as upool, \
             tc.tile_pool(name="tmpp", bufs=2) as tmpp, \
             tc.tile_pool(name="smp", bufs=2) as smp, \
             tc.tile_pool(name="tr1p", bufs=2) as tr1p, \
             tc.tile_pool(name="tr2p", bufs=2) as tr2p, \
             tc.tile_pool(name="tr3p", bufs=2) as tr3p, \
             tc.tile_pool(name="t4p", bufs=2) as t4p, \
             tc.tile_pool(name="bstate", bufs=1) as bstate, \
             tc.tile_pool(name="small", bufs=4) as small, \
             tc.tile_pool(name="psu", bufs=2, space="PSUM") as psu, \
             tc.tile_pool(name="psacc", bufs=1, space="PSUM") as psacc:

            ones_sb = constp.tile([128, 32], f16)
            nc.sync.dma_start(ones_sb[:], ones4[:])
            xbd_sb = xw.tile([128, NP, 128], f16)
            nc.sync.dma_start(xbd_sb[:], x_bd[:])
            x8_sb = xw.tile([128, NP, B], f16)
            nc.scalar.dma_start(x8_sb[:], x8[:])
            orep = [constp.tile([128, DN], f16, tag=f"orep{r}",
                                name=f"orep{r}") for r in range(2)]

            def scale_to_orep(orep_t, cc_out_list, pre_scale):
                """orep <- s * fac with fac[b,n] chosen so that
                orep = squash(pre_scale*s)*INV_LOG2, all on 128-part
                tiles; s loaded 4-replicated from the AR output(s)."""
                s_rep = sqp.tile([128, DN], f16, tag="q_srep")
                for j in range(4):
                    nc.scalar.dma_start(s_rep[32 * j:32 * j + 32, :],
                                        cc_out_list[0][:])
                if len(cc_out_list) > 1:
                    s_rep2 = sqp.tile([128, DN], f16, tag="q_srep2")
                    for j in range(4):
                        nc.scalar.dma_start(s_rep2[32 * j:32 * j + 32, :],
                                            cc_out_list[1][:])
                    s_sum = sqp.tile([128, DN], f16, tag="q_ssum")
                    nc.vector.tensor_add(s_sum[:], s_rep[:], s_rep2[:])
                    s_rep = s_sum
                # sq = (pre*s)^2 via the ACT affine (keeps fp16 in range)
                sq = sqp.tile([128, D, N], f16, tag="q_sq")
                nc.scalar.activation(
                    sq[:], s_rep[:].rearrange("p (d n) -> p d n", n=N),
                    AF.Square, bias=0.0, scale=float(pre_scale))
                cur, d = sq, D
                while d > 1:
                    nxt = sqp.tile([128, d // 2, N], f16, tag=f"q_t{d}")
                    nc.vector.tensor_add(nxt[:], cur[:, 0:d // 2, :],
                                         cur[:, d // 2:d, :])
                    cur, d = nxt, d // 2
                r_ = sqp.tile([128, N], f32, tag="q_r")
                nc.scalar.activation(r_[:], cur[:, 0, :], AF.Sqrt,
                                     bias=0.0, scale=1.0)
                den = sqp.tile([128, N], f32, tag="q_den")
                nc.vector.tensor_scalar(den[:], cur[:, 0, :], 1.0, 1.0,
                                        OP.mult, OP.add)
                rd = sqp.tile([128, N], f32, tag="q_rd")
                nc.vector.reciprocal(rd[:], den[:])
                fac = sqp.tile([128, N], f16, tag="q_fac")
                nc.vector.scalar_tensor_tensor(fac[:], r_[:],
                                               float(pre_scale * INV_LOG2),
                                               rd[:],
                                               op0=OP.mult, op1=OP.mult)
                nc.vector.tensor_mul(
                    orep_t[:].rearrange("p (d n) -> p d n", n=N),
                    s_rep[:].rearrange("p (d n) -> p d n", n=N),
                    fac[:].unsqueeze(1).broadcast_to([128, D, N]))

            # =========== pass 1: s0 directly from x,W ===========
            s0_ps = psacc.tile([B, DN], f32, tag="sacc")
            for gp in range(NP):
                wt = wp.tile([128, DN], f16, tag="wt")
                nc.sync.dma_start(wt[:], w_t2[gp])
                for ch in range(4):
                    nc.tensor.matmul(
                        s0_ps[:, 512 * ch:512 * (ch + 1)],
                        lhsT=x8_sb[:, gp, :],
                        rhs=wt[:, 512 * ch:512 * (ch + 1)],
                        start=(gp == 0), stop=(gp == NP - 1),
                        skip_group_check=True)
            s0_dr = drp.tile([B, DN], f16, tag="t_io16")
            nc.scalar.copy(s0_dr[:], s0_ps[:])
            nc.scalar.dma_start(cc_in[0][:], s0_dr[:])
            nc.gpsimd.collective_compute(
                "AllReduce", OP.add, ins=[cc_in[0][:]],
                outs=[cc_out[0][:]], replica_groups=[core_ids])
            scale_to_orep(orep[0], [cc_out[0]], 1.0 / 64.0)

            # =========== merged: u_hat production (pass 2) + iter 1 ===========
            pair_tiles = {}    # pair id -> sbuf tile [128, 2, DN]
            bs_tiles = {}

            def pair_tile(gp):
                if gp < RES_P:
                    return upool.tile([128, 2, DN], f16, tag=f"ur{gp}",
                                      name=f"ur{gp}", bufs=1)
                return upool.tile([128, 2, DN], f16, tag="rot", name="rot",
                                  bufs=ROT)

            def emit_produce(gp):
                """Pass-2 W load + u_hat matmuls + drains for pair gp."""
                wt = wp.tile([128, DN], f16, tag="wt")
                nc.sync.dma_start(wt[:], w_t2[gp])
                pt = pair_tile(gp)
                pair_tiles[gp] = pt
                for gs in range(2):
                    for h in range(2):
                        pu = psu.tile([128, DN // 2], f32)
                        for ch in range(2):
                            nc.tensor.matmul(
                                pu[:, 512 * ch:512 * (ch + 1)],
                                lhsT=xbd_sb[64 * gs:64 * (gs + 1), gp, :],
                                rhs=wt[64 * gs:64 * (gs + 1),
                                       1024 * h + 512 * ch:
                                       1024 * h + 512 * (ch + 1)],
                                start=True, stop=True)
                        nc.scalar.copy(pt[:, gs, 1024 * h:1024 * (h + 1)],
                                       pu[:])
                if gp >= RES_P:
                    nc.sync.dma_start(
                        u_spill[2 * gp:2 * gp + 2].transpose([1, 0, 2]), pt[:])

            def load_pair(q):
                """DRAM -> rotating tile for spilled pair q (iter 2)."""
                pt = upool.tile([128, 2, DN], f16, tag="rot", name="rot",
                                bufs=ROT)
                nc.sync.dma_start(
                    pt[:], u_spill[2 * q:2 * q + 2].transpose([1, 0, 2]))
                return pt

            st = {}

            def it_stage_a(q, u_pt, orep_t):
                """tmp mul + DVE tree stages (d 32 -> 4)."""
                u3 = u_pt[:]                                  # [128, 2, 2048]
                orep3 = orep_t[:].unsqueeze(1).broadcast_to([128, 2, DN])
                tmp = tmpp.tile([128, 2, DN], f16, tag="tmp")
                nc.vector.tensor_mul(tmp[:], u3, orep3)
                t1 = tr1p.tile([128, 2, 16 * N], f16, tag="t1")
                nc.vector.tensor_add(t1[:], tmp[:, :, 0:16 * N],
                                     tmp[:, :, 16 * N:32 * N])
                t2 = tr2p.tile([128, 2, 8 * N], f16, tag="t2")
                nc.vector.tensor_add(t2[:], t1[:, :, 0:8 * N],
                                     t1[:, :, 8 * N:16 * N])
                t3 = tr3p.tile([128, 2, 4 * N], f16, tag="t3")
                nc.vector.tensor_add(t3[:], t2[:, :, 0:4 * N],
                                     t2[:, :, 4 * N:8 * N])
                st[q] = {"u3": u3, "t3": t3}

            def it_stage_b(q, first_iter):
                """DVE tree tail + bs update + max; ACT exp."""
                t3 = st[q]["t3"]
                t4 = t4p.tile([128, 2, 2 * N], f16, tag="t4")
                nc.vector.tensor_add(t4[:], t3[:, :, 0:2 * N],
                                     t3[:, :, 2 * N:4 * N])
                if first_iter:
                    bs = bstate.tile([128, 2, N], f32, tag=f"bs{q}",
                                     name=f"bs{q}")
                    bs_tiles[q] = bs
                    nc.vector.tensor_add(bs[:], t4[:, :, 0:N], t4[:, :, N:2 * N])
                else:
                    bs = bs_tiles[q]
                    a2 = small.tile([128, 2, N], f32, tag="a2")
                    nc.vector.tensor_add(a2[:], t4[:, :, 0:N], t4[:, :, N:2 * N])
                    nc.vector.tensor_add(bs[:], bs[:], a2[:])
                nm2 = small.tile([128, 2, 1], f32, tag="nm2")
                nc.vector.tensor_reduce(out=nm2[:], in_=bs[:], axis=AX.X,
                                        op=OP.max, negate=True)
                e2 = small.tile([128, 2, N], f16, tag="e2")
                for gs in range(2):
                    nc.scalar.activation(e2[:, gs, :], bs[:, gs, :], AF.Exp,
                                         bias=nm2[:, gs, :], scale=1.0)
                st[q].update(e2=e2)

            def it_stage_c(q, s_ps, first_mm, last_mm):
                """DVE rz + sm mul; ACT cz4; PE folds."""
                d = st.pop(q)
                z2 = small.tile([128, 2, 1], f32, tag="z2")
                nc.vector.tensor_reduce(out=z2[:], in_=d["e2"][:], axis=AX.X,
                                        op=OP.add)
                rz = small.tile([128, 2, 1], f32, tag="rz")
                nc.vector.reciprocal(rz[:], z2[:])
                cz4 = small.tile([128, 2, 32], f16, tag="cz4")
                for gs in range(2):
                    nc.scalar.activation(cz4[:, gs, :], ones_sb[:], AF.Copy,
                                         bias=0.0, scale=rz[:, gs, :])
                sm = smp.tile([128, 2, DN], f16, tag="sm")
                u4 = d["u3"].rearrange("p a (d n) -> p a d n", n=N)
                e4 = d["e2"][:].unsqueeze(2).broadcast_to([128, 2, D, N])
                nc.vector.tensor_mul(
                    sm[:].rearrange("p a (d n) -> p a d n", n=N), u4, e4)
                for gs in range(2):
                    for ch in range(4):
                        nc.tensor.matmul(
                            s_ps[:, 512 * ch:512 * (ch + 1)],
                            lhsT=cz4[:, gs, :],
                            rhs=sm[:, gs, 512 * ch:512 * (ch + 1)],
                            start=(first_mm and gs == 0),
                            stop=(last_mm and gs == 1),
                            skip_group_check=True)

            s1_ps = psacc.tile([B, DN], f32, tag="sacc")
            for step in range(NP + LAG + 2):
                if step < NP:
                    emit_produce(step)
                pa = step - LAG
                if 0 <= pa < NP:
                    it_stage_a(pa, pair_tiles[pa], orep[0])
                pb = pa - 1
                if 0 <= pb < NP:
                    it_stage_b(pb, first_iter=True)
                pc = pa - 2
                if 0 <= pc < NP:
                    it_stage_c(pc, s1_ps, first_mm=(pc == 0),
                               last_mm=(pc == NP - 1))
                    if pc >= RES_P:
                        pair_tiles.pop(pc)

            # iter-1 tail: AllReduce -> orep1
            s1_dr = drp.tile([B, DN], f16, tag="t_io16")
            nc.scalar.copy(s1_dr[:], s1_ps[:])
            nc.scalar.dma_start(cc_in[1][:], s1_dr[:])
            nc.gpsimd.collective_compute(
                "AllReduce", OP.add, ins=[cc_in[1][:]],
                outs=[cc_out[1][:]], replica_groups=[core_ids])
            # prefetch first spilled pairs for iter 2 during the AR
            it2_tiles = {}
            for w in range(RES_P, RES_P + 3):
                it2_tiles[w] = load_pair(w)
            scale_to_orep(orep[1], [cc_out[1]], 1.0)

            # =========== iter 2 (2-stage emission skew) ===========
            s2_ps = psacc.tile([B, DN], f32, tag="sacc")
            for step in range(NP + 2):
                pa = step
                if pa < NP:
                    for w in range(max(pa, RES_P), min(pa + 3, NP)):
                        if w not in it2_tiles:
                            it2_tiles[w] = load_pair(w)
                    if pa < RES_P:
                        u_pt = pair_tiles[pa]
                    else:
                        u_pt = it2_tiles.pop(pa)
                    it_stage_a(pa, u_pt, orep[1])
                pb = step - 1
                if 0 <= pb < NP:
                    it_stage_b(pb, first_iter=False)
                pc = step - 2
                if 0 <= pc < NP:
                    it_stage_c(pc, s2_ps, first_mm=(pc == 0),
                               last_mm=(pc == NP - 1))

            s2_dr = drp.tile([B, DN], f32, tag="t_io")
            nc.scalar.copy(s2_dr[:], s2_ps[:])
            nc.sync.dma_start(s2_part[:], s2_dr[:])

    nc.compile()
    return nc


_NC_CACHE = {}


def _get_nc():
    if "nc" not in _NC_CACHE:
        _NC_CACHE["nc"] = _build()
    return _NC_CACHE["nc"]


def _prep_core(x_c, w_c):
    """x_c [B, IL, K] f32, w_c [N, IL, D, K] f32 -> in_map dict."""
    wt = np.ascontiguousarray(w_c.transpose(1, 3, 2, 0))  # [IL, K, D, N]
    wt2 = wt.reshape(NP, 8, K, DN).reshape(NP, 128, DN).astype(np.float16)
    xt = x_c.transpose(1, 2, 0)  # [IL, K, B]
    x_bd = np.zeros((128, NP, 128), np.float16)
    for g in range(G):
        q, s = g // 2, g % 2
        for j in range(4):
            i = 4 * g + j
            x_bd[s * 64 + j * 16:s * 64 + j * 16 + K, q,
                 j * 32:j * 32 + 32] = xt[i].astype(np.float16)
    # x8[(i8,k), gp, b] = x[b, i, k] for i = gp*8 + i8
    x8 = np.ascontiguousarray(
        xt.reshape(NP, 8 * K, B).transpose(1, 0, 2)).astype(np.float16)
    return {"w_t2": wt2, "x_bd": x_bd, "x8": x8}


def _squash_np(v):
    sn = np.sum(v * v, axis=-1, keepdims=True)
    return np.sqrt(sn) / (1.0 + sn) * v


def _run(inputs, W, trace=False):
    _install_ntff_hook()
    nc = _get_nc()
    x = np.asarray(inputs, np.float32)
    Wf = np.asarray(W, np.float32)
    in_maps = []
    for c in range(NCORES):
        sl = slice(c * IL, (c + 1) * IL)
        in_maps.append(_prep_core(x[:, sl, :], Wf[:, sl, :, :]))
    res = run_bass_kernel_spmd(nc, in_maps, list(range(NCORES)), trace=trace)
    s2 = np.zeros((B, DN), np.float64)
    for c in range(NCORES):
        s2 += res.results[c]["s2_part"].astype(np.float64)
    s2 = s2.reshape(B, D, N).transpose(0, 2, 1).astype(np.float32)
    out = _squash_np(s2).astype(np.float32)
    return out, res


def kernel(inputs, W):
    out, _ = _run(inputs, W, trace=False)
    return out


# revision 31
# speedup vs baseline: 1.2885x; 1.0470x over previous
"""Capsule-routing kernel for 8 Trainium2 NeuronCores.

Problem: u_hat = einsum('nidk,bik->bnid', W, x); 3 rounds of dynamic
routing (softmax over n, weighted sum over i, squash, agreement update).

Sharding: input-capsule axis i (2048) split 8 ways -> 256 i per core.
Per-iteration partial sums s[b,n,d] are combined with an on-device
fp16 AllReduce (iters 0,1) or on the host (final iteration).

v2 design (from ~723us baseline):
  - s0 (iteration-0 weighted sum; c is uniform) is computed DIRECTLY
    from x,W on PE: s0[b,(d,n)] = sum_{i,k} x8[(i8,k),b]*W[(i8,k),(d,n)].
  - W is streamed twice (pass 1 for s0, pass 2 for u_hat).
  - Iter 1 consumes u_hat pairs DIRECTLY from the production SBUF tiles
    (unified pair pool: RES_P pinned + ROT rotating slots); the spill to
    DRAM happens write-behind and is only read back for iter 2.
  - squash is never materialized on device: v = squash(s)*INV_LOG2 =
    s * fac[b,n], so we load s 4-replicated ([128,dn]) from the AR
    output and compute fac with a short 128-partition tree + smalls,
    then one broadcast mul -> orep.  (~5us vs ~13us for the old
    32-partition full squash; also kills the frep/o16 ops.)
  - Latency-critical DMAs (cc_in writes, s_rep loads) ride the Scalar
    HWDGE ring, away from the bulk W/u traffic on the Sync ring.
  - All elementwise work on DVE plain tensor_tensor fp16 (2x mode);
    agreement = tmp mul + 4-stage halving tree over d; psum drains on
    ACT; softmax max negated via tensor_reduce feeds exp bias; fold via
    cz4 ones-matmul accumulated across groups into one psum tile.
  - Layouts: u16 partition p = 32*j + b (j = i mod 4 in group), free
    (d,n) d-major.
"""
import sys
import types

sys.path.insert(0, "/opt/trn_rl_repo")

import numpy as np

from concourse import bacc, tile, mybir
from concourse.bass_utils import run_bass_kernel_spmd

f32 = mybir.dt.float32
f16 = mybir.dt.float16
f8 = mybir.dt.float8e4
AX = mybir.AxisListType
OP = mybir.AluOpType
AF = mybir.ActivationFunctionType

B, N, I, D, K = 32, 64, 2048, 32, 16
NCORES = 8
IL = I // NCORES          # 256 local input capsules
G = IL // 4               # 64 groups of 4 i
NP = G // 2               # 32 group-pairs
DN = D * N                # 2048 free elements per group, d-major
INV_LOG2 = float(1.0 / np.log(2.0))

RES_P = 2                 # pairs kept SBUF-resident; rest spilled to DRAM
ROT = 7                   # rotating pair slots
LAG = 2                   # iter1 consumes pair q at step q+LAG


def _install_ntff_hook():
    if "antenv.axon_hooks" in sys.modules:
        return
    try:
        mod = types.ModuleType("antenv.axon_hooks")
        state = {"hook": None}
        mod.set_axon_ntff_profile_hook = lambda h: state.__setitem__("hook", h)
        mod.get_axon_ntff_profile_hook = lambda: state["hook"]
        sys.modules["antenv.axon_hooks"] = mod
        import antenv
        antenv.axon_hooks = mod
        from trn_agent_boot.trn_boot import _ntff_profile_via_ctypes
        mod.set_axon_ntff_profile_hook(
            _ntff_profile_via_ctypes("/opt/axon/libaxon_pjrt.so"))
    except Exception:
        pass


def _build():
    nc = bacc.Bacc("TRN2", target_bir_lowering=False, debug=False,
                   num_devices=NCORES)

    w_t2 = nc.dram_tensor("w_t2", [NP, 128, DN], f16, kind="ExternalInput")
    x_bd = nc.dram_tensor("x_bd", [128, NP, 128], f16, kind="ExternalInput")
    x8 = nc.dram_tensor("x8", [128, NP, B], f16, kind="ExternalInput")
    s2_part = nc.dram_tensor("s2_part", [B, DN], f32, kind="ExternalOutput")

    u_spill = nc.dram_tensor("u_spill", [G, 128, DN], f16)
    cc_in = [nc.dram_tensor(f"cc_in{r}", [B, DN], f16) for r in range(2)]
    cc_out = [nc.dram_tensor(f"cc_out{r}", [B, DN], f16, addr_space="Shared")
              for r in range(2)]
    cc_din = nc.dram_tensor("cc_din", [32, 2], f16)
    cc_dout = nc.dram_tensor("cc_dout", [32, 2], f16, addr_space="Shared")

    ones4_np = np.zeros((128, 32), np.float16)
    for p in range(128):
        ones4_np[p, p % 32] = 1.0
    ones4 = nc.inline_tensor(ones4_np, name="ones4")

    core_ids = list(range(NCORES))

    with tile.TileContext(nc) as tc:
        with tc.tile_pool(name="const", bufs=1) as constp, \
             tc.tile_pool(name="sqp", bufs=1) as sqp, \
             tc.tile_pool(name="drp", bufs=1) as drp, \
             tc.tile_pool(name="xw", bufs=1) as xw, \
             tc.tile_pool(name="wp", bufs=4) as wp, \
             tc.tile_pool(name="upool", bufs=1) as upool, \
             tc.tile_pool(name="tmpp", bufs=2) as tmpp, \
             tc.tile_pool(name="smp", bufs=2) as smp, \
             tc.tile_pool(name="tr1p", bufs=2) as tr1p, \
             tc.tile_pool(name="tr2p", bufs=2) as tr2p, \
             tc.tile_pool(name="tr3p", bufs=2) as tr3p, \
             tc.tile_pool(name="t4p", bufs=2) as t4p, \
             tc.tile_pool(name="bstate", bufs=1) as bstate, \
             tc.tile_pool(name="small", bufs=4) as small, \
             tc.tile_pool(name="psu", bufs=2, space="PSUM") as psu, \
             tc.tile_pool(name="psacc", bufs=1, space="PSUM") as psacc:

            # dummy collective at t=0: prepays the ~60us first-collective
            # arm cost + absorbs launch skew, so the s0 AR runs warm
            nc.gpsimd.collective_compute(
                "AllReduce", OP.add, ins=[cc_din[:]],
                outs=[cc_dout[:]], replica_groups=[core_ids])
            ones_sb = constp.tile([128, 32], f16)
            nc.sync.dma_start(ones_sb[:], ones4[:])
            xbd_sb = xw.tile([128, NP, 128], f16)
            nc.sync.dma_start(xbd_sb[:], x_bd[:])
            x8_sb = xw.tile([128, NP, B], f16)
            nc.scalar.dma_start(x8_sb[:], x8[:])
            orep = [constp.tile([128, DN], f16, tag=f"orep{r}",
                                name=f"orep{r}") for r in range(2)]

            def scale_to_orep(orep_t, cc_out_list, pre_scale):
                """orep <- s * fac with fac[b,n] chosen so that
                orep = squash(pre_scale*s)*INV_LOG2, all on 128-part
                tiles; s loaded 4-replicated from the AR output(s)."""
                s_rep = sqp.tile([128, DN], f16, tag="q_srep")
                for j in range(4):
                    nc.scalar.dma_start(s_rep[32 * j:32 * j + 32, :],
                                        cc_out_list[0][:])
                if len(cc_out_list) > 1:
                    s_rep2 = sqp.tile([128, DN], f16, tag="q_srep2")
                    for j in range(4):
                        nc.scalar.dma_start(s_rep2[32 * j:32 * j + 32, :],
                                            cc_out_list[1][:])
                    s_sum = sqp.tile([128, DN], f16, tag="q_ssum")
                    nc.vector.tensor_add(s_sum[:], s_rep[:], s_rep2[:])
                    s_rep = s_sum
                # sq = (pre*s)^2 via the ACT affine (keeps fp16 in range)
                sq = sqp.tile([128, D, N], f16, tag="q_sq")
                nc.scalar.activation(
                    sq[:], s_rep[:].rearrange("p (d n) -> p d n", n=N),
                    AF.Square, bias=0.0, scale=float(pre_scale))
                cur, d = sq, D
                while d > 1:
                    nxt = sqp.tile([128, d // 2, N], f16, tag=f"q_t{d}")
                    nc.vector.tensor_add(nxt[:], cur[:, 0:d // 2, :],
                                         cur[:, d // 2:d, :])
                    cur, d = nxt, d // 2
                r_ = sqp.tile([128, N], f32, tag="q_r")
                nc.scalar.activation(r_[:], cur[:, 0, :], AF.Sqrt,
                                     bias=0.0, scale=1.0)
                den = sqp.tile([128, N], f32, tag="q_den")
                nc.vector.tensor_scalar(den[:], cur[:, 0, :], 1.0, 1.0,
                                        OP.mult, OP.add)
                rd = sqp.tile([128, N], f32, tag="q_rd")
                nc.vector.reciprocal(rd[:], den[:])
                fac = sqp.tile([128, N], f16, tag="q_fac")
                nc.vector.scalar_tensor_tensor(fac[:], r_[:],
                                               float(pre_scale * INV_LOG2),
                                               rd[:],
                                               op0=OP.mult, op1=OP.mult)
                nc.vector.tensor_mul(
                    orep_t[:].rearrange("p (d n) -> p d n", n=N),
                    s_rep[:].rearrange("p (d n) -> p d n", n=N),
                    fac[:].unsqueeze(1).broadcast_to([128, D, N]))

            # =========== pass 1: s0 directly from x,W ===========
            s0_ps = psacc.tile([B, DN], f32, tag="sacc")
            for gp in range(NP):
                wt = wp.tile([128, DN], f16, tag="wt")
                nc.sync.dma_start(wt[:], w_t2[gp])
                for ch in range(4):
                    nc.tensor.matmul(
                        s0_ps[:, 512 * ch:512 * (ch + 1)],
                        lhsT=x8_sb[:, gp, :],
                        rhs=wt[:, 512 * ch:512 * (ch + 1)],
                        start=(gp == 0), stop=(gp == NP - 1),
                        skip_group_check=True)
            s0_dr = drp.tile([B, DN], f16, tag="t_io16")
            nc.scalar.copy(s0_dr[:], s0_ps[:])
            nc.scalar.dma_start(cc_in[0][:], s0_dr[:])
            nc.gpsimd.collective_compute(
                "AllReduce", OP.add, ins=[cc_in[0][:]],
                outs=[cc_out[0][:]], replica_groups=[core_ids])
            scale_to_orep(orep[0], [cc_out[0]], 1.0 / 64.0)

            # =========== merged: u_hat production (pass 2) + iter 1 ===========
            pair_tiles = {}    # pair id -> sbuf tile [128, 2, DN]
            bs_tiles = {}

            def pair_tile(gp):
                if gp < RES_P:
                    return upool.tile([128, 2, DN], f16, tag=f"ur{gp}",
                                      name=f"ur{gp}", bufs=1)
                return upool.tile([128, 2, DN], f16, tag="rot", name="rot",
                                  bufs=ROT)

            def emit_produce(gp):
                """Pass-2 W load + u_hat matmuls + drains for pair gp."""
                wt = wp.tile([128, DN], f16, tag="wt")
                nc.sync.dma_start(wt[:], w_t2[gp])
                pt = pair_tile(gp)
                pair_tiles[gp] = pt
                for gs in range(2):
                    for h in range(2):
                        pu = psu.tile([128, DN // 2], f32)
                        for ch in range(2):
                            nc.tensor.matmul(
                                pu[:, 512 * ch:512 * (ch + 1)],
                                lhsT=xbd_sb[64 * gs:64 * (gs + 1), gp, :],
                                rhs=wt[64 * gs:64 * (gs + 1),
                                       1024 * h + 512 * ch:
                                       1024 * h + 512 * (ch + 1)],
                                start=True, stop=True)
                        nc.scalar.copy(pt[:, gs, 1024 * h:1024 * (h + 1)],
                                       pu[:])
                if gp >= RES_P:
                    nc.scalar.dma_start(
                        u_spill[2 * gp:2 * gp + 2].transpose([1, 0, 2]), pt[:])

            def load_pair(q):
                """DRAM -> rotating tile for spilled pair q (iter 2)."""
                pt = upool.tile([128, 2, DN], f16, tag="rot", name="rot",
                                bufs=ROT)
                nc.sync.dma_start(
                    pt[:], u_spill[2 * q:2 * q + 2].transpose([1, 0, 2]))
                return pt

            st = {}

            def it_stage_a(q, u_pt, orep_t):
                """tmp mul + DVE tree stages (d 32 -> 4)."""
                u3 = u_pt[:]                                  # [128, 2, 2048]
                orep3 = orep_t[:].unsqueeze(1).broadcast_to([128, 2, DN])
                tmp = tmpp.tile([128, 2, DN], f16, tag="tmp")
                nc.vector.tensor_mul(tmp[:], u3, orep3)
                t1 = tr1p.tile([128, 2, 16 * N], f16, tag="t1")
                nc.vector.tensor_add(t1[:], tmp[:, :, 0:16 * N],
                                     tmp[:, :, 16 * N:32 * N])
                t2 = tr2p.tile([128, 2, 8 * N], f16, tag="t2")
                nc.vector.tensor_add(t2[:], t1[:, :, 0:8 * N],
                                     t1[:, :, 8 * N:16 * N])
                t3 = tr3p.tile([128, 2, 4 * N], f16, tag="t3")
                nc.vector.tensor_add(t3[:], t2[:, :, 0:4 * N],
                                     t2[:, :, 4 * N:8 * N])
                st[q] = {"u3": u3, "t3": t3}

            def it_stage_b(q, first_iter):
                """DVE tree tail + bs update + max; ACT exp."""
                t3 = st[q]["t3"]
                t4 = t4p.tile([128, 2, 2 * N], f16, tag="t4")
                nc.vector.tensor_add(t4[:], t3[:, :, 0:2 * N],
                                     t3[:, :, 2 * N:4 * N])
                if first_iter:
                    bs = bstate.tile([128, 2, N], f32, tag=f"bs{q}",
                                     name=f"bs{q}")
                    bs_tiles[q] = bs
                    nc.vector.tensor_add(bs[:], t4[:, :, 0:N], t4[:, :, N:2 * N])
                else:
                    bs = bs_tiles[q]
                    a2 = small.tile([128, 2, N], f32, tag="a2")
                    nc.vector.tensor_add(a2[:], t4[:, :, 0:N], t4[:, :, N:2 * N])
                    nc.vector.tensor_add(bs[:], bs[:], a2[:])
                nm2 = small.tile([128, 2, 1], f32, tag="nm2")
                nc.vector.tensor_reduce(out=nm2[:], in_=bs[:], axis=AX.X,
                                        op=OP.max, negate=True)
                e2 = small.tile([128, 2, N], f16, tag="e2")
                for gs in range(2):
                    nc.scalar.activation(e2[:, gs, :], bs[:, gs, :], AF.Exp,
                                         bias=nm2[:, gs, :], scale=1.0)
                st[q].update(e2=e2)

            def it_stage_c(q, s_ps, first_mm, last_mm):
                """DVE rz + sm mul; ACT cz4; PE folds."""
                d = st.pop(q)
                z2 = small.tile([128, 2, 1], f32, tag="z2")
                nc.vector.tensor_reduce(out=z2[:], in_=d["e2"][:], axis=AX.X,
                                        op=OP.add)
                rz = small.tile([128, 2, 1], f32, tag="rz")
                nc.vector.reciprocal(rz[:], z2[:])
                cz4 = small.tile([128, 2, 32], f16, tag="cz4")
                for gs in range(2):
                    nc.scalar.activation(cz4[:, gs, :], ones_sb[:], AF.Copy,
                                         bias=0.0, scale=rz[:, gs, :])
                sm = smp.tile([128, 2, DN], f16, tag="sm")
                u4 = d["u3"].rearrange("p a (d n) -> p a d n", n=N)
                e4 = d["e2"][:].unsqueeze(2).broadcast_to([128, 2, D, N])
                nc.vector.tensor_mul(
                    sm[:].rearrange("p a (d n) -> p a d n", n=N), u4, e4)
                for gs in range(2):
                    for ch in range(4):
                        nc.tensor.matmul(
                            s_ps[:, 512 * ch:512 * (ch + 1)],
                            lhsT=cz4[:, gs, :],
                            rhs=sm[:, gs, 512 * ch:512 * (ch + 1)],
                            start=(first_mm and gs == 0),
                            stop=(last_mm and gs == 1),
                            skip_group_check=True)

            s1_ps = psacc.tile([B, DN], f32, tag="sacc")
            for step in range(NP + LAG + 2):
                if step < NP:
                    emit_produce(step)
                pa = step - LAG
                if 0 <= pa < NP:
                    it_stage_a(pa, pair_tiles[pa], orep[0])
                pb = pa - 1
                if 0 <= pb < NP:
                    it_stage_b(pb, first_iter=True)
                pc = pa - 2
                if 0 <= pc < NP:
                    it_stage_c(pc, s1_ps, first_mm=(pc == 0),
                               last_mm=(pc == NP - 1))
                    if pc >= RES_P:
                        pair_tiles.pop(pc)

            # iter-1 tail: AllReduce -> orep1
            s1_dr = drp.tile([B, DN], f16, tag="t_io16")
            nc.scalar.copy(s1_dr[:], s1_ps[:])
            nc.scalar.dma_start(cc_in[1][:], s1_dr[:])
            nc.gpsimd.collective_compute(
                "AllReduce", OP.add, ins=[cc_in[1][:]],
                outs=[cc_out[1][:]], replica_groups=[core_ids])
            # prefetch first spilled pairs for iter 2 during the AR
            it2_tiles = {}
            for w in range(RES_P, RES_P + 3):
                it2_tiles[w] = load_pair(w)
            scale_to_orep(orep[1], [cc_out[1]], 1.0)

            # =========== iter 2 (2-stage emission skew) ===========
            s2_ps = psacc.tile([B, DN], f32, tag="sacc")
            for step in range(NP + 2):
                pa = step
                if pa < NP:
                    for w in range(max(pa, RES_P), min(pa + 3, NP)):
                        if w not in it2_tiles:
                            it2_tiles[w] = load_pair(w)
                    if pa < RES_P:
                        u_pt = pair_tiles[pa]
                    else:
                        u_pt = it2_tiles.pop(pa)
                    it_stage_a(pa, u_pt, orep[1])
                pb = step - 1
                if 0 <= pb < NP:
                    it_stage_b(pb, first_iter=False)
                pc = step - 2
                if 0 <= pc < NP:
                    it_stage_c(pc, s2_ps, first_mm=(pc == 0),
                               last_mm=(pc == NP - 1))

            s2_dr = drp.tile([B, DN], f32, tag="t_io")
            nc.scalar.copy(s2_dr[:], s2_ps[:])
            nc.sync.dma_start(s2_part[:], s2_dr[:])

    nc.compile()
    return nc


_NC_CACHE = {}


def _get_nc():
    if "nc" not in _NC_CACHE:
        _NC_CACHE["nc"] = _build()
    return _NC_CACHE["nc"]


def _prep_core(x_c, w_c):
    """x_c [B, IL, K] f32, w_c [N, IL, D, K] f32 -> in_map dict."""
    wt = np.ascontiguousarray(w_c.transpose(1, 3, 2, 0))  # [IL, K, D, N]
    wt2 = wt.reshape(NP, 8, K, DN).reshape(NP, 128, DN).astype(np.float16)
    xt = x_c.transpose(1, 2, 0)  # [IL, K, B]
    x_bd = np.zeros((128, NP, 128), np.float16)
    for g in range(G):
        q, s = g // 2, g % 2
        for j in range(4):
            i = 4 * g + j
            x_bd[s * 64 + j * 16:s * 64 + j * 16 + K, q,
                 j * 32:j * 32 + 32] = xt[i].astype(np.float16)
    # x8[(i8,k), gp, b] = x[b, i, k] for i = gp*8 + i8
    x8 = np.ascontiguousarray(
        xt.reshape(NP, 8 * K, B).transpose(1, 0, 2)).astype(np.float16)
    return {"w_t2": wt2, "x_bd": x_bd, "x8": x8}


def _squash_np(v):
    sn = np.sum(v * v, axis=-1, keepdims=True)
    return np.sqrt(sn) / (1.0 + sn) * v


def _run(inputs, W, trace=False):
    _install_ntff_hook()
    nc = _get_nc()
    x = np.asarray(inputs, np.float32)
    Wf = np.asarray(W, np.float32)
    in_maps = []
    for c in range(NCORES):
        sl = slice(c * IL, (c + 1) * IL)
        in_maps.append(_prep_core(x[:, sl, :], Wf[:, sl, :, :]))
    res = run_bass_kernel_spmd(nc, in_maps, list(range(NCORES)), trace=trace)
    s2 = np.zeros((B, DN), np.float64)
    for c in range(NCORES):
        s2 += res.results[c]["s2_part"].astype(np.float64)
    s2 = s2.reshape(B, D, N).transpose(0, 2, 1).astype(np.float32)
    out = _squash_np(s2).astype(np.float32)
    return out, res


def kernel(inputs, W):
    out, _ = _run(inputs, W, trace=False)
    return out
